# revision 31
# baseline (speedup 1.0000x reference)
"""JacobiGNN Trainium2 kernel: out = log_softmax(U @ (H * (U^T z)), axis=1).

v6: bf16 datapath, no AllGather. Every core computes the full MLP z = [h|1] @
[W2; b2] replicated (x is small), with z landing directly row-partitioned in
SBUF -- no transposes, no z collective, so the only collective is the final
ReduceScatter, issued late enough that the collective engine's ~60-85us async
init is already done. Column-shard U across 8 cores (1024 spectral cols each).
Per 128-col block b: each 128x128 U tile is loaded once as PE weights; we
stream z (16 cols, accumulating G_b = U_b^T z in f32 PSUM) and the identity
(128 cols, materializing the transposed tile -> SBUF bf16). GEMM2 then uses
the transposed tiles as weights and streams y_b = H_b * G_b (16 cols),
accumulating output rows in natural layout in PSUM across blocks (PSUM
start=True pends-zero its whole 2KB bank, so only the first write per bank
sets it). A single bf16 ReduceScatter over the partition-major flat buffer
sums partials across cores; log_softmax runs on the local [16, 64, 16] shard.
"""

import os
import sys

import numpy as np

for _p in ("/opt/trn_rl_repo", "/root/.axon_site/_ro/trn_rl_repo"):
    if os.path.isdir(_p) and _p not in sys.path:
        sys.path.insert(0, _p)

import ml_dtypes

import concourse.bacc as bacc
import concourse.bass as bass  # noqa: F401
import concourse.mybir as mybir
import concourse.tile as tile
from concourse.bass_utils import run_bass_kernel_spmd

F32 = mybir.dt.float32
BF16 = mybir.dt.bfloat16
NPBF16 = ml_dtypes.bfloat16
N, F_IN, HID, C, K = 8192, 512, 64, 16, 10
BASE_ALPHA = 0.5
JA, JB, JL, JR = 1.0, 1.0, -1.0, 1.0
NCORES = 8
SH = N // NCORES      # spectral columns per core (1024)
NB = SH // 128        # column blocks per core (8)
RCH = N // 128        # row chunks (64)
MYR = SH // 128       # (8)
NQ = 4                # MLP row quarters
QR = N // NQ          # rows per quarter (2048)

_CACHE = {}


def _jacobi_coef_rows(temp):
    """Host-precomputed per-channel coefficient rows, [30*C] packed."""
    a, b, l, r = JA, JB, JL, JR
    alphas = (BASE_ALPHA * np.tanh(np.asarray(temp, np.float64)))  # [C, K+1]
    rows = [alphas[:, 0]]
    coef1 = (a - b) / 2 - (a + b + 2) / 2 * (l + r) / (r - l)
    coef2 = (a + b + 2) / (r - l)
    rows.append(coef1 * alphas[:, 1])   # c1_0
    rows.append(coef2 * alphas[:, 1])   # c1_1
    for L in range(2, K + 1):
        coef_l = 2 * L * (L + a + b) * (2 * L - 2 + a + b)
        c_lm1_1 = (2 * L + a + b - 1) * (2 * L + a + b) * (2 * L + a + b - 2)
        c_lm1_2 = (2 * L + a + b - 1) * (a ** 2 - b ** 2)
        c_lm2 = 2 * (L - 1 + a) * (L - 1 + b) * (2 * L + a + b)
        tmp1 = alphas[:, L - 1] * (c_lm1_1 / coef_l)
        tmp2 = alphas[:, L - 1] * (c_lm1_2 / coef_l)
        tmp3 = alphas[:, L - 1] * alphas[:, L - 2] * (c_lm2 / coef_l)
        rows.append(tmp1 * (2 / (r - l)))                    # t1
        rows.append(tmp1 * ((r + l) / (r - l)) + tmp2)       # t2
        rows.append(tmp3)                                    # t3
    packed = np.concatenate(rows).astype(np.float32).reshape(1, 30 * C)
    return np.ascontiguousarray(np.repeat(packed, 128, axis=0))


def _bc(ap, shape, axis=1):
    """Broadcast an AP to a 3D [128, NB, C]-style shape with stride-0 dims."""
    while ap.ndim < len(shape):
        ap = ap.unsqueeze(axis)
    return ap.broadcast_to(shape)


def _build():
    nc = bacc.Bacc("TRN2", target_bir_lowering=False, debug=False)

    u4 = nc.dram_tensor("u4", [NB, 128, RCH, 128], BF16, kind="ExternalInput")
    x3 = nc.dram_tensor("x3", [128, 4, N], BF16, kind="ExternalInput")
    e_sh = nc.dram_tensor("e_shard", [MYR, 128], F32, kind="ExternalInput")
    w1r = nc.dram_tensor("w1r", [128, 4 * HID], BF16, kind="ExternalInput")
    w2aug = nc.dram_tensor("w2aug", [HID + 1, C], BF16, kind="ExternalInput")
    b1c = nc.dram_tensor("b1c", [HID, 1], F32, kind="ExternalInput")
    jcd = nc.dram_tensor("jcd", [128, 30 * C], F32, kind="ExternalInput")
    id128d = nc.dram_tensor("id128d", [128, 128], BF16, kind="ExternalInput")
    id16d = nc.dram_tensor("id16d", [C, C], F32, kind="ExternalInput")
    onesd = nc.dram_tensor("onesd", [1, N], BF16, kind="ExternalInput")
    out_sh = nc.dram_tensor("out_sh", [C, RCH, C], F32, kind="ExternalOutput")

    rg = [list(range(NCORES))]

    with nc.allow_low_precision(reason="bf16 matmul path, tol 2e-2"), \
         tile.TileContext(nc) as tc:
        with (
            tc.tile_pool(name="dram", bufs=1, space="DRAM") as dram,
            tc.tile_pool(name="consts", bufs=1) as cp,
            tc.tile_pool(name="persist", bufs=1) as pp,
            tc.tile_pool(name="xsb", bufs=2) as xp,
            tc.tile_pool(name="usb", bufs=3) as up,
            tc.tile_pool(name="utsb", bufs=3) as utp,
            tc.tile_pool(name="small", bufs=4) as sp,
            tc.tile_pool(name="ysb", bufs=4) as yp,
        ):
            rs_in = dram.tile([128, RCH, C], BF16)
            rs_out = dram.tile([C, RCH, C], BF16)

            id128 = cp.tile_from(id128d[:])
            id16 = cp.tile_from(id16d[:])
            jc = cp.tile_from(jcd[:])
            w1 = cp.tile_from(w1r[:])
            w2a = cp.tile_from(w2aug[:])
            b1 = cp.tile_from(b1c[:])
            e_row = cp.tile_from(e_sh[:])

            # ---- persistent SBUF ----
            h_aug = pp.tile([HID + 1, N], BF16)      # [relu(xW1+b1) | 1]^T
            zid = pp.tile([128, RCH, C], BF16)       # full z, chunk-major
            e_col = pp.tile([128, MYR], F32)
            hacc = pp.tile([128, NB, C], F32)        # Jacobi filter H
            xs_a = pp.tile([128, NB, C], F32)
            xs_b = pp.tile([128, NB, C], F32)
            htmp = pp.tile([128, NB, C], F32)
            htmp2 = pp.tile([128, NB, C], F32)
            out_sb = pp.tile([128, RCH, C], BF16)
            srs = pp.tile([C, RCH, C], BF16)
            smt = pp.tile([C, RCH, C], F32)
            smet = pp.tile([C, RCH, C], F32)
            smout = pp.tile([C, RCH, C], F32)

            # u block prefetch on the sync HWDGE ring
            u_tiles = []
            for b in range(2):
                u_t = up.tile([128, RCH, 128], BF16, tag="u")
                nc.sync.dma_start(out=u_t[:], in_=u4[b])
                u_tiles.append(u_t)

            # ========== phase 0: replicated MLP -> z, row-partitioned ======
            with tc.tile_pool(name="ppre", bufs=1, space="PSUM") as ppre:
                nc.scalar.dma_start(out=h_aug[HID:HID + 1, :], in_=onesd[:])
                xqs = []

                def dma_x(q):
                    xq = xp.tile([128, 4, QR], BF16, tag="xq")
                    nc.scalar.dma_start(out=xq[:], in_=x3[:, :, q * QR:(q + 1) * QR])
                    xqs.append(xq)

                def emit_ph(q):
                    for piece in range(QR // 512):
                        ph = ppre.tile([HID, 512], F32, tag="ph", bufs=2)
                        for fb in range(4):
                            nc.tensor.matmul(
                                ph[:], lhsT=w1[:, fb * HID:(fb + 1) * HID],
                                rhs=xqs[q][:, fb, piece * 512:(piece + 1) * 512],
                                start=(fb == 0), stop=(fb == 3),
                            )
                        lo = q * QR + piece * 512
                        nc.scalar.activation(
                            h_aug[0:HID, lo:lo + 512], ph[:],
                            mybir.ActivationFunctionType.Relu,
                            bias=b1[:, 0:1], scale=1.0)

                def emit_z(q):
                    for grp in range(4):
                        pzc = ppre.tile([128, 4, C], F32, tag="pzc", bufs=4)
                        for j in range(4):
                            rc = q * 16 + grp * 4 + j
                            nc.tensor.matmul(
                                pzc[:, j, :],
                                lhsT=h_aug[:, rc * 128:(rc + 1) * 128],
                                rhs=w2a[:], start=True, stop=True,
                            )
                        nc.vector.tensor_copy(
                            zid[:, (q * 16 + grp * 4):(q * 16 + grp * 4 + 4), :],
                            pzc[:])

                dma_x(0)
                dma_x(1)
                emit_ph(0)
                dma_x(2)
                emit_ph(1)
                dma_x(3)
                emit_z(0)
                emit_ph(2)
                emit_z(1)
                emit_ph(3)
                emit_z(2)
                emit_z(3)
                # e: [8, 128] -> [128, 8]
                pet = ppre.tile([128, MYR], F32, tag="ph", bufs=2)
                nc.tensor.transpose(pet[:], e_row[:], id16[0:MYR, 0:MYR])
                nc.scalar.copy(e_col[:], pet[:])

            # ================= Jacobi filter H on DVE ====================
            ev = _bc(e_col[:], (128, NB, C), axis=2)

            def jrow(i):
                return _bc(jc[:, i * C:(i + 1) * C], (128, NB, C))

            nc.vector.tensor_copy(xs_a[:], jrow(0))                       # xs_m2
            nc.vector.tensor_mul(htmp[:], xs_a[:], ev)
            nc.vector.tensor_mul(htmp[:], htmp[:], jrow(2))
            nc.vector.tensor_add(xs_b[:], htmp[:], jrow(1))               # xs_m1
            nc.vector.tensor_add(hacc[:], xs_a[:], xs_b[:])
            xm2, xm1 = xs_a, xs_b
            for L in range(2, K + 1):
                r0 = 3 + 3 * (L - 2)
                nc.vector.tensor_mul(htmp[:], xm1[:], ev)
                nc.vector.tensor_mul(htmp[:], htmp[:], jrow(r0))
                nc.vector.tensor_mul(htmp2[:], xm1[:], jrow(r0 + 1))
                nc.vector.tensor_sub(htmp[:], htmp[:], htmp2[:])
                nc.vector.tensor_mul(htmp2[:], xm2[:], jrow(r0 + 2))
                nc.vector.tensor_sub(xm2[:], htmp[:], htmp2[:])           # nx
                nc.vector.tensor_add(hacc[:], hacc[:], xm2[:])
                xm2, xm1 = xm1, xm2

            # ================= main loop over column blocks ==============
            with tc.tile_pool(name="pmain", bufs=1, space="PSUM") as pm:
                oacc = pm.tile([128, RCH, C], F32, tag="oacc")   # 2 banks
                ut_tiles = {}

                def gemm1_both(b, u_t):
                    """Interleaved transpose + z-accumulate sweep (shared weights)."""
                    ut_sb = utp.tile([128, RCH, 128], BF16, tag="ut")
                    g_ps = pm.tile([128, C], F32, tag="g", bufs=2)
                    for g in range(RCH // 8):
                        pt = pm.tile([128, 8, 128], F32, tag="pt", bufs=2)
                        for j in range(8):
                            rc = 8 * g + j
                            nc.tensor.matmul(
                                g_ps[:], lhsT=u_t[:, rc, :], rhs=zid[:, rc, :],
                                start=(rc == 0), stop=(rc == RCH - 1),
                                skip_group_check=True,
                            )
                            nc.tensor.matmul(
                                pt[:, j, :], lhsT=u_t[:, rc, :], rhs=id128[:],
                                start=True, stop=True,
                            )
                        dst = ut_sb[:, 8 * g:8 * (g + 1), :]
                        if g % 2 == 0:
                            nc.scalar.copy(dst, pt[:])
                        else:
                            nc.vector.tensor_copy(dst, pt[:])
                    ut_tiles[b] = ut_sb
                    y_sb = yp.tile([128, C], BF16, tag="y")
                    nc.vector.tensor_mul(y_sb[:], g_ps[:], hacc[:, b, :])
                    return y_sb

                def gemm2(b, y_sb):
                    """out rows += Ut_b^T y_b, natural layout, accumulate over b."""
                    ut_sb = ut_tiles.pop(b)
                    for rc in range(RCH):
                        # start=True pends-zero the WHOLE 2KB psum zero region
                        # (bank); only the first write per bank may set it.
                        nc.tensor.matmul(
                            oacc[:, rc, :], lhsT=ut_sb[:, rc, :], rhs=y_sb[:],
                            start=(b == 0 and rc % 32 == 0), stop=(b == NB - 1),
                            skip_group_check=True,
                        )

                ys = {}
                for b in range(NB):
                    if b + 2 < NB:
                        u_t = up.tile([128, RCH, 128], BF16, tag="u")
                        nc.sync.dma_start(out=u_t[:], in_=u4[b + 2])
                        u_tiles.append(u_t)
                    ys[b] = gemm1_both(b, u_tiles[b])
                    if b > 0:
                        gemm2(b - 1, ys.pop(b - 1))
                gemm2(NB - 1, ys.pop(NB - 1))

                # flush -> bf16 -> ReduceScatter on partition-major buffer
                nc.scalar.copy(out_sb[:, 0:RCH // 2, :], oacc[:, 0:RCH // 2, :])
                nc.vector.tensor_copy(out_sb[:, RCH // 2:, :], oacc[:, RCH // 2:, :])
                nc.sync.dma_start(out=rs_in[:], in_=out_sb[:])
            nc.gpsimd.collective_compute(
                "ReduceScatter", mybir.AluOpType.add, replica_groups=rg,
                ins=[rs_in.opt()], outs=[rs_out.opt()],
            )
            # log_softmax on the local [16, 64, 16] shard: single Exp + single
            # Ln (one activation-table load each), per-row stats via DVE.
            nc.sync.dma_start(out=srs[:], in_=rs_out[:])
            mneg = sp.tile([C, RCH, 1], F32, tag="mneg")
            ssum = sp.tile([C, RCH, 1], F32, tag="ssum")
            lns = sp.tile([C, RCH, 1], F32, tag="lns")
            nc.vector.tensor_reduce(out=mneg[:], in_=srs[:], op=mybir.AluOpType.max,
                                    axis=mybir.AxisListType.X, negate=True)
            nc.vector.tensor_add(smt[:], srs[:], mneg[:].broadcast_to((C, RCH, C)))
            nc.scalar.activation(smet[:], smt[:], mybir.ActivationFunctionType.Exp)
            nc.vector.tensor_reduce(out=ssum[:], in_=smet[:], op=mybir.AluOpType.add,
                                    axis=mybir.AxisListType.X)
            nc.scalar.activation(lns[:], ssum[:], mybir.ActivationFunctionType.Ln)
            nc.vector.tensor_sub(smout[:], smt[:], lns[:].broadcast_to((C, RCH, C)))
            nc.scalar.dma_start(out=out_sh[:], in_=smout[:])

    nc.compile()
    return nc


def _prep_inputs(origin_e, U, x, W1, b1, W2, b2, temp):
    origin_e = np.ascontiguousarray(np.asarray(origin_e, np.float32))
    U = np.asarray(U, np.float32)
    x = np.asarray(x, np.float32)
    W1 = np.asarray(W1, np.float32)
    b1 = np.asarray(b1, np.float32)
    W2 = np.asarray(W2, np.float32)
    b2 = np.asarray(b2, np.float32)

    jc = _jacobi_coef_rows(temp)
    id128 = np.eye(128, dtype=NPBF16)
    id16 = np.eye(C, dtype=np.float32)
    w1r = np.ascontiguousarray(
        W1.reshape(4, 128, HID).transpose(1, 0, 2).reshape(128, 4 * HID)
        .astype(NPBF16))
    w2aug = np.ascontiguousarray(
        np.concatenate([W2, b2.reshape(1, C)], axis=0).astype(NPBF16))
    xb = x.astype(NPBF16)
    # x3[p, a, r] = x[r, a*128 + p]
    x3 = np.ascontiguousarray(xb.T.reshape(4, 128, N).transpose(1, 0, 2))
    shared = {
        "w1r": w1r, "w2aug": w2aug,
        "b1c": np.ascontiguousarray(b1.reshape(HID, 1)),
        "jcd": jc, "id128d": id128, "id16d": id16,
        "onesd": np.ones((1, N), dtype=NPBF16), "x3": x3,
    }
    Ub = U.astype(NPBF16)
    in_maps = []
    for i in range(NCORES):
        m = dict(shared)
        # u4[b, p, rc, c] = U[rc*128 + p, i*1024 + b*128 + c]
        A = Ub[:, i * SH:(i + 1) * SH]              # [8192, 1024]
        A = A.reshape(RCH, 128, NB, 128)            # [rc, p, b, c]
        m["u4"] = np.ascontiguousarray(A.transpose(2, 1, 0, 3))
        m["e_shard"] = np.ascontiguousarray(
            origin_e[i * SH:(i + 1) * SH].reshape(MYR, 128))
        in_maps.append(m)
    return in_maps


def _get_program():
    if "nc" not in _CACHE:
        _CACHE["nc"] = _build()
    return _CACHE["nc"]


def _unshard(res):
    """res[i]['out_sh'] is [16, 64, 16] with row = rc*128 + 16*i + p."""
    M = np.empty((RCH, 128, C), dtype=np.float32)
    for i in range(NCORES):
        arr = np.asarray(res.results[i]["out_sh"], np.float32)
        M[:, C * i:C * (i + 1), :] = arr.transpose(1, 0, 2)
    return M.reshape(N, C)


def run(inputs, trace=False, **kw):
    nc = _get_program()
    in_maps = _prep_inputs(**inputs)
    res = run_bass_kernel_spmd(nc, in_maps, core_ids=list(range(NCORES)),
                               trace=trace, **kw)
    return _unshard(res), res


def kernel(origin_e, U, x, W1, b1, W2, b2, temp):
    out, _ = run(dict(origin_e=origin_e, U=U, x=x, W1=W1, b1=b1, W2=W2,
                      b2=b2, temp=temp))
    return out


# revision 36
# speedup vs baseline: 1.1311x; 1.1311x over previous
"""JacobiGNN Trainium2 kernel: out = log_softmax(U @ (H * (U^T z)), axis=1).

v6: bf16 datapath, no AllGather. Every core computes the full MLP z = [h|1] @
[W2; b2] replicated (x is small), with z landing directly row-partitioned in
SBUF -- no transposes, no z collective, so the only collective is the final
ReduceScatter, issued late enough that the collective engine's ~60-85us async
init is already done. Column-shard U across 8 cores (1024 spectral cols each).
Per 128-col block b: each 128x128 U tile is loaded once as PE weights; we
stream z (16 cols, accumulating G_b = U_b^T z in f32 PSUM) and the identity
(128 cols, materializing the transposed tile -> SBUF bf16). GEMM2 then uses
the transposed tiles as weights and streams y_b = H_b * G_b (16 cols),
accumulating output rows in natural layout in PSUM across blocks (PSUM
start=True pends-zero its whole 2KB bank, so only the first write per bank
sets it). A single bf16 ReduceScatter over the partition-major flat buffer
sums partials across cores; log_softmax runs on the local [16, 64, 16] shard.
"""

import os
import sys

import numpy as np

for _p in ("/opt/trn_rl_repo", "/root/.axon_site/_ro/trn_rl_repo"):
    if os.path.isdir(_p) and _p not in sys.path:
        sys.path.insert(0, _p)

import ml_dtypes

import concourse.bacc as bacc
import concourse.bass as bass  # noqa: F401
import concourse.mybir as mybir
import concourse.tile as tile
from concourse.bass_utils import run_bass_kernel_spmd

F32 = mybir.dt.float32
BF16 = mybir.dt.bfloat16
NPBF16 = ml_dtypes.bfloat16
N, F_IN, HID, C, K = 8192, 512, 64, 16, 10
BASE_ALPHA = 0.5
JA, JB, JL, JR = 1.0, 1.0, -1.0, 1.0
NCORES = 8
SH = N // NCORES      # spectral columns per core (1024)
NB = SH // 128        # column blocks per core (8)
RCH = N // 128        # row chunks (64)
MYR = SH // 128       # (8)
NQ = 4                # MLP row quarters
QR = N // NQ          # rows per quarter (2048)

_CACHE = {}


def _jacobi_coef_rows(temp):
    """Host-precomputed per-channel coefficient rows, [30*C] packed."""
    a, b, l, r = JA, JB, JL, JR
    alphas = (BASE_ALPHA * np.tanh(np.asarray(temp, np.float64)))  # [C, K+1]
    rows = [alphas[:, 0]]
    coef1 = (a - b) / 2 - (a + b + 2) / 2 * (l + r) / (r - l)
    coef2 = (a + b + 2) / (r - l)
    rows.append(coef1 * alphas[:, 1])   # c1_0
    rows.append(coef2 * alphas[:, 1])   # c1_1
    for L in range(2, K + 1):
        coef_l = 2 * L * (L + a + b) * (2 * L - 2 + a + b)
        c_lm1_1 = (2 * L + a + b - 1) * (2 * L + a + b) * (2 * L + a + b - 2)
        c_lm1_2 = (2 * L + a + b - 1) * (a ** 2 - b ** 2)
        c_lm2 = 2 * (L - 1 + a) * (L - 1 + b) * (2 * L + a + b)
        tmp1 = alphas[:, L - 1] * (c_lm1_1 / coef_l)
        tmp2 = alphas[:, L - 1] * (c_lm1_2 / coef_l)
        tmp3 = alphas[:, L - 1] * alphas[:, L - 2] * (c_lm2 / coef_l)
        rows.append(tmp1 * (2 / (r - l)))                    # t1
        rows.append(tmp1 * ((r + l) / (r - l)) + tmp2)       # t2
        rows.append(tmp3)                                    # t3
    packed = np.concatenate(rows).astype(np.float32).reshape(1, 30 * C)
    return np.ascontiguousarray(np.repeat(packed, 128, axis=0))


def _bc(ap, shape, axis=1):
    """Broadcast an AP to a 3D [128, NB, C]-style shape with stride-0 dims."""
    while ap.ndim < len(shape):
        ap = ap.unsqueeze(axis)
    return ap.broadcast_to(shape)


def _build():
    nc = bacc.Bacc("TRN2", target_bir_lowering=False, debug=False)

    u4 = nc.dram_tensor("u4", [NB, 128, RCH, 128], BF16, kind="ExternalInput")
    x3 = nc.dram_tensor("x3", [128, 4, N], BF16, kind="ExternalInput")
    e_sh = nc.dram_tensor("e_shard", [MYR, 128], F32, kind="ExternalInput")
    w1r = nc.dram_tensor("w1r", [128, 4 * HID], BF16, kind="ExternalInput")
    w2aug = nc.dram_tensor("w2aug", [HID + 1, C], BF16, kind="ExternalInput")
    b1c = nc.dram_tensor("b1c", [HID, 1], F32, kind="ExternalInput")
    jcd = nc.dram_tensor("jcd", [128, 30 * C], F32, kind="ExternalInput")
    id128d = nc.dram_tensor("id128d", [128, 128], BF16, kind="ExternalInput")
    id16d = nc.dram_tensor("id16d", [C, C], F32, kind="ExternalInput")
    onesd = nc.dram_tensor("onesd", [1, N], BF16, kind="ExternalInput")
    out_sh = nc.dram_tensor("out_sh", [C, RCH, C], F32, kind="ExternalOutput")

    rg = [list(range(NCORES))]

    with nc.allow_low_precision(reason="bf16 matmul path, tol 2e-2"), \
         tile.TileContext(nc) as tc:
        with (
            tc.tile_pool(name="dram", bufs=1, space="DRAM") as dram,
            tc.tile_pool(name="consts", bufs=1) as cp,
            tc.tile_pool(name="persist", bufs=1) as pp,
            tc.tile_pool(name="xsb", bufs=2) as xp,
            tc.tile_pool(name="usb", bufs=3) as up,
            tc.tile_pool(name="utsb", bufs=3) as utp,
            tc.tile_pool(name="small", bufs=4) as sp,
            tc.tile_pool(name="ysb", bufs=4) as yp,
        ):
            rs_in = dram.tile([128, RCH, C], BF16)
            rs_out = dram.tile([C, RCH, C], BF16)

            id128 = cp.tile_from(id128d[:])
            id16 = cp.tile_from(id16d[:])
            jc = cp.tile_from(jcd[:])
            w1 = cp.tile_from(w1r[:])
            w2a = cp.tile_from(w2aug[:])
            b1 = cp.tile_from(b1c[:])
            e_row = cp.tile_from(e_sh[:])

            # ---- persistent SBUF ----
            h_aug = pp.tile([HID + 1, N], BF16)      # [relu(xW1+b1) | 1]^T
            zid = pp.tile([128, RCH, C], BF16)       # full z, chunk-major
            e_col = pp.tile([128, MYR], F32)
            hacc = pp.tile([128, NB, C], F32)        # Jacobi filter H
            xs_a = pp.tile([128, NB, C], F32)
            xs_b = pp.tile([128, NB, C], F32)
            htmp = pp.tile([128, NB, C], F32)
            htmp2 = pp.tile([128, NB, C], F32)
            out_sb = pp.tile([128, RCH, C], BF16)
            srs = pp.tile([C, RCH, C], BF16)
            smt = pp.tile([C, RCH, C], F32)
            smet = pp.tile([C, RCH, C], F32)
            smout = pp.tile([C, RCH, C], F32)

            # All large DMAs share the sync HWDGE ring in priority order: the
            # MLP's x quarters lead (z gates the main loop), u blocks are
            # interleaved to land just-in-time.
            u_tiles = []

            def dma_u(b):
                u_t = up.tile([128, RCH, 128], BF16, tag="u")
                nc.sync.dma_start(out=u_t[:], in_=u4[b])
                u_tiles.append(u_t)

            # ========== phase 0: replicated MLP -> z, row-partitioned ======
            with tc.tile_pool(name="ppre", bufs=1, space="PSUM") as ppre:
                nc.scalar.dma_start(out=h_aug[HID:HID + 1, :], in_=onesd[:])
                xqs = []

                def dma_x(q):
                    xq = xp.tile([128, 4, QR], BF16, tag="xq")
                    nc.sync.dma_start(out=xq[:], in_=x3[:, :, q * QR:(q + 1) * QR])
                    xqs.append(xq)

                def emit_ph(q):
                    for piece in range(QR // 512):
                        ph = ppre.tile([HID, 512], F32, tag="ph", bufs=2)
                        for fb in range(4):
                            nc.tensor.matmul(
                                ph[:], lhsT=w1[:, fb * HID:(fb + 1) * HID],
                                rhs=xqs[q][:, fb, piece * 512:(piece + 1) * 512],
                                start=(fb == 0), stop=(fb == 3),
                            )
                        lo = q * QR + piece * 512
                        nc.scalar.activation(
                            h_aug[0:HID, lo:lo + 512], ph[:],
                            mybir.ActivationFunctionType.Relu,
                            bias=b1[:, 0:1], scale=1.0)

                def emit_z(q):
                    for grp in range(4):
                        pzc = ppre.tile([128, 4, C], F32, tag="pzc", bufs=4)
                        for j in range(4):
                            rc = q * 16 + grp * 4 + j
                            nc.tensor.matmul(
                                pzc[:, j, :],
                                lhsT=h_aug[:, rc * 128:(rc + 1) * 128],
                                rhs=w2a[:], start=True, stop=True,
                            )
                        nc.vector.tensor_copy(
                            zid[:, (q * 16 + grp * 4):(q * 16 + grp * 4 + 4), :],
                            pzc[:])

                dma_x(0)
                dma_x(1)
                emit_ph(0)
                dma_x(2)
                emit_ph(1)
                dma_x(3)
                dma_u(0)
                emit_z(0)
                emit_ph(2)
                dma_u(1)
                emit_z(1)
                emit_ph(3)
                emit_z(2)
                emit_z(3)
                # e: [8, 128] -> [128, 8]
                pet = ppre.tile([128, MYR], F32, tag="ph", bufs=2)
                nc.tensor.transpose(pet[:], e_row[:], id16[0:MYR, 0:MYR])
                nc.scalar.copy(e_col[:], pet[:])

            # ================= Jacobi filter H on DVE ====================
            ev = _bc(e_col[:], (128, NB, C), axis=2)

            def jrow(i):
                return _bc(jc[:, i * C:(i + 1) * C], (128, NB, C))

            nc.vector.tensor_copy(xs_a[:], jrow(0))                       # xs_m2
            nc.vector.tensor_mul(htmp[:], xs_a[:], ev)
            nc.vector.tensor_mul(htmp[:], htmp[:], jrow(2))
            nc.vector.tensor_add(xs_b[:], htmp[:], jrow(1))               # xs_m1
            nc.vector.tensor_add(hacc[:], xs_a[:], xs_b[:])
            xm2, xm1 = xs_a, xs_b
            for L in range(2, K + 1):
                r0 = 3 + 3 * (L - 2)
                nc.vector.tensor_mul(htmp[:], xm1[:], ev)
                nc.vector.tensor_mul(htmp[:], htmp[:], jrow(r0))
                nc.vector.tensor_mul(htmp2[:], xm1[:], jrow(r0 + 1))
                nc.vector.tensor_sub(htmp[:], htmp[:], htmp2[:])
                nc.vector.tensor_mul(htmp2[:], xm2[:], jrow(r0 + 2))
                nc.vector.tensor_sub(xm2[:], htmp[:], htmp2[:])           # nx
                nc.vector.tensor_add(hacc[:], hacc[:], xm2[:])
                xm2, xm1 = xm1, xm2

            # ================= main loop over column blocks ==============
            with tc.tile_pool(name="pmain", bufs=1, space="PSUM") as pm:
                oacc = pm.tile([128, RCH, C], F32, tag="oacc")   # 2 banks
                ut_tiles = {}

                def gemm1_both(b, u_t):
                    """Interleaved transpose + z-accumulate sweep (shared weights)."""
                    ut_sb = utp.tile([128, RCH, 128], BF16, tag="ut")
                    g_ps = pm.tile([128, C], F32, tag="g", bufs=2)
                    for g in range(RCH // 8):
                        pt = pm.tile([128, 8, 128], F32, tag="pt", bufs=2)
                        for j in range(8):
                            rc = 8 * g + j
                            nc.tensor.matmul(
                                g_ps[:], lhsT=u_t[:, rc, :], rhs=zid[:, rc, :],
                                start=(rc == 0), stop=(rc == RCH - 1),
                                skip_group_check=True,
                            )
                            nc.tensor.matmul(
                                pt[:, j, :], lhsT=u_t[:, rc, :], rhs=id128[:],
                                start=True, stop=True,
                            )
                        dst = ut_sb[:, 8 * g:8 * (g + 1), :]
                        if g % 2 == 0:
                            nc.scalar.copy(dst, pt[:])
                        else:
                            nc.vector.tensor_copy(dst, pt[:])
                    ut_tiles[b] = ut_sb
                    y_sb = yp.tile([128, C], BF16, tag="y")
                    nc.vector.tensor_mul(y_sb[:], g_ps[:], hacc[:, b, :])
                    return y_sb

                def gemm2(b, y_sb):
                    """out rows += Ut_b^T y_b, natural layout, accumulate over b."""
                    ut_sb = ut_tiles.pop(b)
                    for rc in range(RCH):
                        # start=True pends-zero the WHOLE 2KB psum zero region
                        # (bank); only the first write per bank may set it.
                        nc.tensor.matmul(
                            oacc[:, rc, :], lhsT=ut_sb[:, rc, :], rhs=y_sb[:],
                            start=(b == 0 and rc % 32 == 0), stop=(b == NB - 1),
                            skip_group_check=True,
                        )

                ys = {}
                for b in range(NB):
                    if b + 2 < NB:
                        dma_u(b + 2)
                    ys[b] = gemm1_both(b, u_tiles[b])
                    if b > 0:
                        gemm2(b - 1, ys.pop(b - 1))
                gemm2(NB - 1, ys.pop(NB - 1))

                # flush -> bf16 -> ReduceScatter on partition-major buffer
                nc.scalar.copy(out_sb[:, 0:RCH // 2, :], oacc[:, 0:RCH // 2, :])
                nc.vector.tensor_copy(out_sb[:, RCH // 2:, :], oacc[:, RCH // 2:, :])
                nc.sync.dma_start(out=rs_in[:], in_=out_sb[:])
            nc.gpsimd.collective_compute(
                "ReduceScatter", mybir.AluOpType.add, replica_groups=rg,
                ins=[rs_in.opt()], outs=[rs_out.opt()],
            )
            # log_softmax on the local [16, 64, 16] shard: single Exp + single
            # Ln (one activation-table load each), per-row stats via DVE.
            nc.sync.dma_start(out=srs[:], in_=rs_out[:])
            mneg = sp.tile([C, RCH, 1], F32, tag="mneg")
            ssum = sp.tile([C, RCH, 1], F32, tag="ssum")
            lns = sp.tile([C, RCH, 1], F32, tag="lns")
            nc.vector.tensor_reduce(out=mneg[:], in_=srs[:], op=mybir.AluOpType.max,
                                    axis=mybir.AxisListType.X, negate=True)
            nc.vector.tensor_add(smt[:], srs[:], mneg[:].broadcast_to((C, RCH, C)))
            nc.scalar.activation(smet[:], smt[:], mybir.ActivationFunctionType.Exp)
            nc.vector.tensor_reduce(out=ssum[:], in_=smet[:], op=mybir.AluOpType.add,
                                    axis=mybir.AxisListType.X)
            nc.scalar.activation(lns[:], ssum[:], mybir.ActivationFunctionType.Ln)
            nc.vector.tensor_sub(smout[:], smt[:], lns[:].broadcast_to((C, RCH, C)))
            nc.scalar.dma_start(out=out_sh[:], in_=smout[:])

    nc.compile()
    return nc


def _prep_inputs(origin_e, U, x, W1, b1, W2, b2, temp):
    origin_e = np.ascontiguousarray(np.asarray(origin_e, np.float32))
    U = np.asarray(U, np.float32)
    x = np.asarray(x, np.float32)
    W1 = np.asarray(W1, np.float32)
    b1 = np.asarray(b1, np.float32)
    W2 = np.asarray(W2, np.float32)
    b2 = np.asarray(b2, np.float32)

    jc = _jacobi_coef_rows(temp)
    id128 = np.eye(128, dtype=NPBF16)
    id16 = np.eye(C, dtype=np.float32)
    w1r = np.ascontiguousarray(
        W1.reshape(4, 128, HID).transpose(1, 0, 2).reshape(128, 4 * HID)
        .astype(NPBF16))
    w2aug = np.ascontiguousarray(
        np.concatenate([W2, b2.reshape(1, C)], axis=0).astype(NPBF16))
    xb = x.astype(NPBF16)
    # x3[p, a, r] = x[r, a*128 + p]
    x3 = np.ascontiguousarray(xb.T.reshape(4, 128, N).transpose(1, 0, 2))
    shared = {
        "w1r": w1r, "w2aug": w2aug,
        "b1c": np.ascontiguousarray(b1.reshape(HID, 1)),
        "jcd": jc, "id128d": id128, "id16d": id16,
        "onesd": np.ones((1, N), dtype=NPBF16), "x3": x3,
    }
    Ub = U.astype(NPBF16)
    in_maps = []
    for i in range(NCORES):
        m = dict(shared)
        # u4[b, p, rc, c] = U[rc*128 + p, i*1024 + b*128 + c]
        A = Ub[:, i * SH:(i + 1) * SH]              # [8192, 1024]
        A = A.reshape(RCH, 128, NB, 128)            # [rc, p, b, c]
        m["u4"] = np.ascontiguousarray(A.transpose(2, 1, 0, 3))
        m["e_shard"] = np.ascontiguousarray(
            origin_e[i * SH:(i + 1) * SH].reshape(MYR, 128))
        in_maps.append(m)
    return in_maps


def _get_program():
    if "nc" not in _CACHE:
        _CACHE["nc"] = _build()
    return _CACHE["nc"]


def _unshard(res):
    """res[i]['out_sh'] is [16, 64, 16] with row = rc*128 + 16*i + p."""
    M = np.empty((RCH, 128, C), dtype=np.float32)
    for i in range(NCORES):
        arr = np.asarray(res.results[i]["out_sh"], np.float32)
        M[:, C * i:C * (i + 1), :] = arr.transpose(1, 0, 2)
    return M.reshape(N, C)


def run(inputs, trace=False, **kw):
    nc = _get_program()
    in_maps = _prep_inputs(**inputs)
    res = run_bass_kernel_spmd(nc, in_maps, core_ids=list(range(NCORES)),
                               trace=trace, **kw)
    return _unshard(res), res


def kernel(origin_e, U, x, W1, b1, W2, b2, temp):
    out, _ = run(dict(origin_e=origin_e, U=U, x=x, W1=W1, b1=b1, W2=W2,
                      b2=b2, temp=temp))
    return out
